# revision 31
# baseline (speedup 1.0000x reference)
"""GQA causal attention (RoPE) on 8 Trainium2 NeuronCores.

Sharding (tensor-parallel over heads, per the hint):
  core c owns q-heads {2c, 2c+1} and kv-head c//2.
  Each core computes its 2 heads' attention over the full sequence and a
  partial output projection out_c.T = wo[:, 128c:128c+128] @ att_c  (shape
  [1024, 4096]); the final all-reduce over cores is the host-side unshard.

Device-side per core (v12 — cross-chunk software pipeline):
  The ScalarE exp stream (144 activations, ~1ns/elem, ~151us busy) is the
  binding resource; the group loop of chunk n interleaves, at spread group
  slots, the prologue of chunk n+1 (split projections, rope, v^T) early
  and a deferred endgame (denominator broadcast, normalize, wo, store;
  chunk k's endgame runs in chunk k+2) late, so ScalarE and the PE never
  drain at chunk boundaries and early-chunk DVE pressure is smoothed.

  - All matmuls bf16, fp32 PSUM; scores 2-head row-packed (K=64 tiles),
    h1 emitted first (h0 additionally waits on the krot-duplicate DMA).
  - RoPE from PSUM in fp32: even/odd perm folded into wq/wk host-side,
    sign-folded sin, partition-block swap via SBUF->SBUF DMA (gpsimd ring
    reserved for these latency-critical transfers).
  - Causal masks accumulated on the PE; exp on ScalarE via grouped
    [128, 2, 512] activations with diagonal-trimmed 3D APs (~8% fewer
    elements); PE warmup bursts beat the HAM cold clock at startup.
  - AV with ones-augmented V^T (denominators fall out of the matmul);
    V^T via PE transpose; reciprocal via DVE reciprocal_approx_fast.
  - x / trig / out use chunk-contiguous host layouts -> 1 bulk DMA per
    chunk each (x+out on the sync ring, trig/consts on the scalar ring).

  Measured: 242us HW exec (baseline v4: 346-388us), rel err 3.7e-3.
"""
import numpy as np
import ml_dtypes
from contextlib import ExitStack

import concourse.bacc as bacc
import concourse.tile as tile
import concourse.mybir as mybir
from concourse.bass_utils import run_bass_kernel_spmd

DIM = 1024
N_HEADS = 16
N_KV = 4
HD = 64
SEQ = 4096
NCORES = 8

SQ = 512            # query-chunk (free dim of score blocks)
SK = 128            # key-chunk (partition dim of score blocks)
NQ = SEQ // SQ      # 8
NR = DIM // 128     # 8 contraction chunks for projections
NJ = SEQ // SK      # 32 key chunks
MASKVAL = -1.0e6

f32 = mybir.dt.float32
f32r = mybir.dt.float32r
bf16 = mybir.dt.bfloat16
FT = mybir.ActivationFunctionType

_CACHE = {}


def _emit(nc):
    # chunk-contiguous layouts: x_pre[p, n, r, sq] = x[512n+sq, 128r+p]
    xT = nc.dram_tensor("xT", [128, NQ, NR, SQ], bf16, kind="ExternalInput").ap()
    # trig[p, n, 0, sq] = cos4[p, 512n+sq]; [.., 1, ..] = sin4
    trig_d = nc.dram_tensor("trig", [128, NQ, 2, SQ], bf16, kind="ExternalInput").ap()
    wq_l = nc.dram_tensor("wq_l", [128, DIM], bf16, kind="ExternalInput").ap()
    wkv_l = nc.dram_tensor("wkv_l", [128, DIM], bf16, kind="ExternalInput").ap()
    wo_l = nc.dram_tensor("wo_l", [128, DIM], bf16, kind="ExternalInput").ap()
    mask_d = nc.dram_tensor("mask", [128, 4 * SQ], bf16, kind="ExternalInput").ap()
    id_d = nc.dram_tensor("ident", [128, 128], bf16, kind="ExternalInput").ap()
    # ones column of vt comes from a memset, not a DRAM load
    sel_d = nc.dram_tensor("sel2", [2, 128], f32r, kind="ExternalInput").ap()
    # out[p, n, m, sq] = out_partial[128m+p, 512n+sq] (bf16 partials)
    out_d = nc.dram_tensor("out", [128, NQ, NR, SQ], f32, kind="ExternalOutput").ap()

    with tile.TileContext(nc) as tc, ExitStack() as ctx:
        const = ctx.enter_context(tc.tile_pool(name="const", bufs=1))
        main = ctx.enter_context(tc.tile_pool(name="main", bufs=1))

        wq_sb = const.tile([128, DIM], bf16)
        wkv_sb = const.tile([128, DIM], bf16)
        wo_sb = const.tile([128, DIM], bf16)
        msk_sb = const.tile([128, 4 * SQ], bf16)
        id_sb = const.tile([128, 128], bf16)
        sel_sb = const.tile([2, 128], f32r)

        qrot = main.tile([128, SEQ], bf16)      # 2 heads d-major (rope'd)
        krot = main.tile([128, SEQ], bf16)      # k duplicated in both halves
        v_sb = main.tile([HD, SEQ], bf16)       # v d-major
        vt = main.tile([128, NJ, 128], bf16)    # v^T + ones column (aligned slots)
        attS = main.tile([128, SEQ], bf16)      # stacked normalized att (j-major)
        att1 = main.tile([HD, SEQ], bf16)       # head-1 att staging (lanes 0-63)

        with (
            tc.tile_pool(name="xp", bufs=2) as xp,       # [128, NR*SQ] x chunks
            tc.tile_pool(name="x0p", bufs=1) as x0p,     # chunk-0 split x
            tc.tile_pool(name="tp", bufs=2) as tp,       # trig chunks
            tc.tile_pool(name="pp", bufs=1, space="PSUM") as pp,
            tc.tile_pool(name="rp", bufs=2) as rp,
            tc.tile_pool(name="sp", bufs=2, space="PSUM") as sp,
            tc.tile_pool(name="ap", bufs=1, space="PSUM") as ap,
            tc.tile_pool(name="ep", bufs=4) as ep,
            tc.tile_pool(name="aup", bufs=3) as aup,     # raw AV staging
            tc.tile_pool(name="rbp", bufs=3) as rbp,
            tc.tile_pool(name="op", bufs=1) as op,       # wo-out staging
        ):
            xsb = {}      # n -> x chunk tile (or list of per-r tiles for n=0)
            trg = {}      # n -> trig chunk tile
            au = {}       # n -> (au0, au1, d2)

            def load_x(n):
                t = xp.tile([128, NR, SQ], bf16, tag="x")
                nc.sync.dma_start(t[:], xT[:, n, :, :])
                xsb[n] = lambda r: t[:, r, :]

            def load_x0():
                ts = []
                for r in range(NR):
                    t = x0p.tile([128, SQ], bf16, tag=f"x0_{r}")
                    eng = nc.sync if r % 2 == 0 else nc.scalar
                    eng.dma_start(t[:], xT[:, 0, r, :])
                    ts.append(t)
                xsb[0] = lambda r: ts[r][:]

            def load_trig(n):
                t = tp.tile([128, 2, SQ], bf16, tag="trig")
                nc.scalar.dma_start(t[:], trig_d[:, n, :, :])
                trg[n] = t

            def proj(n, r0=0, r1=NR):
                # pq/pkv accumulate over contraction chunks [r0, r1)
                if r0 == 0:
                    proj.cur = (pp.tile([128, SQ], f32, tag="pq", name=f"pq_{n}"),
                                pp.tile([128, SQ], f32, tag="pkv", name=f"pkv_{n}"))
                pq, pkv = proj.cur
                xt = xsb[n]
                for r in range(r0, r1):
                    nc.tensor.matmul(pq[:], wq_sb[:, 128 * r:128 * (r + 1)],
                                     xt(r), start=(r == 0), stop=(r == NR - 1))
                    nc.tensor.matmul(pkv[:], wkv_sb[:, 128 * r:128 * (r + 1)],
                                     xt(r), start=(r == 0), stop=(r == NR - 1))
                if r1 == NR:
                    xsb.pop(n)
                return proj.cur

            def rope(n, pq, pkv):
                s0 = n * SQ
                trig = trg.pop(n)
                # ---- q ----
                a_t = rp.tile([128, SQ], f32, tag="ta")
                c_t = rp.tile([128, SQ], f32, tag="tc")
                b_t = rp.tile([128, SQ], f32, tag="tb")
                nc.vector.tensor_mul(a_t[:], pq[:], trig[:, 0, :])
                nc.vector.tensor_mul(c_t[:], pq[:], trig[:, 1, :])
                nc.gpsimd.dma_start(b_t[0:32, :], c_t[32:64, :])
                nc.gpsimd.dma_start(b_t[32:64, :], c_t[0:32, :])
                nc.gpsimd.dma_start(b_t[64:96, :], c_t[96:128, :])
                nc.gpsimd.dma_start(b_t[96:128, :], c_t[64:96, :])
                nc.vector.tensor_add(qrot[:, s0:s0 + SQ], a_t[:], b_t[:])
                # ---- k (rows 64:128; v occupies rows 0:64) ----
                ak = rp.tile([128, SQ], f32, tag="ta")
                ck = rp.tile([128, SQ], f32, tag="tc")
                bk = rp.tile([128, SQ], f32, tag="tb")
                nc.vector.tensor_mul(ak[64:128, :], pkv[64:128, :],
                                     trig[64:128, 0, :])
                nc.vector.tensor_mul(ck[64:128, :], pkv[64:128, :],
                                     trig[64:128, 1, :])
                nc.gpsimd.dma_start(bk[64:96, :], ck[96:128, :])
                nc.gpsimd.dma_start(bk[96:128, :], ck[64:96, :])
                nc.vector.tensor_add(krot[64:128, s0:s0 + SQ], ak[64:128, :], bk[64:128, :])
                nc.gpsimd.dma_start(krot[0:64, s0:s0 + SQ], krot[64:128, s0:s0 + SQ])
                # ---- v -> bf16 ----
                nc.vector.tensor_copy(v_sb[:, s0:s0 + SQ], pkv[0:64, :])

            def vtrans(n, half=None):
                # v^T via PE transpose (ping-pong pq/pkv banks)
                js = range(4 * n, 4 * n + 4) if half is None else \
                    range(4 * n + 2 * half, 4 * n + 2 * half + 2)
                for i, j in enumerate(js):
                    tg = "pq" if i % 2 == 0 else "pkv"
                    pt = pp.tile([SK, HD], bf16, tag=tg, name=f"pt_{j}")
                    nc.tensor.transpose(pt[:], v_sb[:, SK * j:SK * (j + 1)],
                                        id_sb[0:HD, 0:HD])
                    nc.vector.tensor_copy(vt[:, j, 0:HD], pt[:])

            def stage_au(n):
                # raw AV + denoms out of PSUM; free av banks
                a0 = aup.tile([HD + 1, SQ], f32r, tag="au0")
                a1 = aup.tile([HD + 1, SQ], f32r, tag="au1")
                nc.vector.tensor_copy(a0[:], avs[n][0][:])
                nc.vector.tensor_copy(a1[:], avs[n][1][:])
                d2 = rbp.tile([2, SQ], f32r, tag="d2")
                nc.gpsimd.dma_start(d2[0:1, :], a0[HD:HD + 1, :])
                nc.gpsimd.dma_start(d2[1:2, :], a1[HD:HD + 1, :])
                au[n] = (a0, a1, d2)

            def endgame_bc(k):
                # denominator broadcast + fast reciprocal + normalize
                sk0 = k * SQ
                a0, a1, d2 = au.pop(k)
                bc = pp.tile([128, SQ], f32, tag="pq", name=f"bc_{k}")
                nc.tensor.matmul(bc[:], sel_sb[:], d2[:], start=True, stop=True)
                rb = rbp.tile([128, SQ], f32, tag="rb")
                nc.vector.reciprocal_approx_fast(rb[:], bc[:])
                rb1 = rbp.tile([HD, SQ], f32, tag="rb1")
                nc.sync.dma_start(rb1[:], rb[64:64 + HD, :])
                nc.vector.tensor_mul(attS[0:HD, sk0:sk0 + SQ],
                                     a0[0:HD, :].bitcast(f32), rb[0:HD, :])
                nc.vector.tensor_mul(att1[:, sk0:sk0 + SQ],
                                     a1[0:HD, :].bitcast(f32), rb1[:])
                nc.sync.dma_start(attS[64:128, sk0:sk0 + SQ], att1[:, sk0:sk0 + SQ])

            def endgame_wo(k, half, wide=False):
                # wo matmuls + evacuation; half 0 -> m 0..3, half 1 -> m 4..7
                sk0 = k * SQ
                if half == 0:
                    endgame_wo.ot = op.tile([128, NR, SQ], f32, tag="ot",
                                            name=f"ot_{k}")
                ot = endgame_wo.ot
                for m in range(4 * half, 4 * half + 4):
                    if wide and m % 2 == 1:
                        pw = sp.tile([128, SQ], f32, tag="sc", name=f"pw_{k}_{m}")
                    else:
                        pw = pp.tile([128, SQ], f32,
                                     tag=("pkv" if m % 2 == 0 else "pq"),
                                     name=f"pw_{k}_{m}")
                    nc.tensor.matmul(pw[:], wo_sb[:, 128 * m:128 * (m + 1)],
                                     attS[:, sk0:sk0 + SQ], start=True, stop=True)
                    nc.vector.tensor_copy(ot[:, m, :], pw[:])
                nc.sync.dma_start(out_d[:, k, 4 * half:4 * half + 4, :],
                                  ot[:, 4 * half:4 * half + 4, :])

            avs = {}

            def attention(n, hooks):
                s0 = n * SQ
                nsk = 4 * (n + 1)
                av = [ap.tile([HD + 1, SQ], f32, tag=f"av{h}", name=f"av{h}_{n}")
                      for h in (0, 1)]
                avs[n] = av
                pend = []   # (j, et, dd) awaiting AV emission

                def flush_av():
                    j_, et_, dd_ = pend.pop(0)
                    for h_ in (0, 1):
                        nc.tensor.matmul(
                            av[h_][:, dd_:SQ], vt[:, j_, 0:HD + 1],
                            et_[:, h_, dd_:SQ],
                            start=(j_ == 0), stop=(j_ == nsk - 1),
                        )

                for gi in range(nsk):
                    j = gi
                    delta = SK * j - s0
                    dd = max(0, delta)
                    sc = sp.tile([128, 2, SQ], f32, tag="sc")
                    for h in (1, 0):
                        nc.tensor.matmul(
                            sc[:, h, dd:SQ],
                            krot[64 * h:64 * h + 64, SK * j:SK * (j + 1)],
                            qrot[64 * h:64 * h + 64, s0 + dd:s0 + SQ],
                            start=True, stop=(delta < 0),
                        )
                        if delta >= 0:
                            db = (delta // SK) * SQ + dd
                            nc.tensor.matmul(sc[:, h, delta:delta + SK],
                                             id_sb[:], msk_sb[:, db:db + SK],
                                             start=False, stop=True)
                    et = ep.tile([128, 2, SQ], bf16, tag="et")
                    with tc.high_priority(offset=100000):
                        nc.scalar.activation(et[:, :, dd:], sc[:, :, dd:],
                                             FT.Exp, scale=0.125)
                    pend.append((j, et, dd))
                    if len(pend) > 2:
                        flush_av()
                    hk = hooks.get(gi)
                    if hk is not None:
                        hk()
                while pend:
                    flush_av()
                stage_au(n)

            # ---------------- program ----------------
            # startup loads (weights first; scalar queue stays clean)
            nc.sync.dma_start(wq_sb[:, 0:128], wq_l[:, 0:128])
            nc.sync.dma_start(wq_sb[:, 128:], wq_l[:, 128:])
            nc.sync.dma_start(wkv_sb[:], wkv_l[:])
            load_x0()
            load_trig(0)
            nc.scalar.dma_start(id_sb[:], id_d[:])
            nc.scalar.dma_start(msk_sb[:], mask_d[:])
            nc.gpsimd.memset(vt[:, :, HD:HD + 1], 1.0)
            nc.scalar.dma_start(sel_sb[:], sel_d[:])
            nc.sync.dma_start(wo_sb[:], wo_l[:])

            # chunk 0 prologue (serial)
            load_x(1)
            # PE warmup: dense tiny matmuls on wq while x0 is in flight,
            # so HAM un-throttles before the real projections start
            wu = pp.tile([128, 128], f32, tag="pq", name="warmup")
            for w in range(32):
                nc.tensor.matmul(wu[:], wq_sb[:, 0:128], wq_sb[:, 0:128],
                                 start=True, stop=True)
            pq, pkv = proj(0)
            wu2 = sp.tile([128, 128], f32, tag="sc", name="warmup2")
            for w in range(14):
                nc.tensor.matmul(wu2[:], wq_sb[:, 0:128], wq_sb[:, 0:128],
                                 start=True, stop=True)
            load_trig(1)
            rope(0, pq, pkv)
            vtrans(0)
            first_prologue = True

            pending_proj = {}

            for n in range(NQ):
                nsk = 4 * (n + 1)
                hooks = {}
                order = []
                # next-chunk prologue early (its outputs gate the next chunk)
                if n + 1 < NQ:
                    def do_proj(k, r0, r1):
                        got = proj(k, r0, r1)
                        if r1 == NR:
                            pending_proj[k] = got
                    def do_rope(k=n + 1):
                        pq_, pkv_ = pending_proj.pop(k)
                        rope(k, pq_, pkv_)
                    order += [lambda k=n + 1: load_x(k + 1) if k + 1 < NQ else None,
                              lambda k=n + 1: do_proj(k, 0, 2),
                              lambda k=n + 1: do_proj(k, 2, 4),
                              lambda k=n + 1: do_proj(k, 4, 6),
                              lambda k=n + 1: do_proj(k, 6, 8),
                              do_rope,
                              lambda k=n + 1: (load_trig(k + 1)
                                               if k + 1 < NQ else None),
                              lambda k=n + 1: vtrans(k, 0),
                              lambda k=n + 1: vtrans(k, 1)]
                # deferred endgames (chunk k+2 for k<=4; chunk 7 gets 5+6)
                for k in {2: [0], 3: [1], 4: [2], 5: [3], 6: [4],
                          7: [5, 6]}.get(n, []):
                    order += [lambda k=k: endgame_bc(k),
                              lambda k=k: endgame_wo(k, 0),
                              lambda k=k: endgame_wo(k, 1)]
                if n == 0:
                    attention(n, {})
                    for fn in order:
                        fn()
                else:
                    # spread across groups (stacking), first hook at gi=1
                    for i, fn in enumerate(order):
                        gi = 1 + (i * (nsk - 1)) // len(order)
                        prev = hooks.get(gi)
                        hooks[gi] = (fn if prev is None else
                                     (lambda a=prev, b=fn: (a(), b())))
                    attention(n, hooks)

            endgame_bc(NQ - 1)
            endgame_wo(NQ - 1, 0, wide=True)
            endgame_wo(NQ - 1, 1, wide=True)


def _build():
    if "nc" in _CACHE:
        return _CACHE["nc"]
    nc = bacc.Bacc("TRN2", target_bir_lowering=False, debug=False, num_devices=NCORES)
    _emit(nc)
    nc.compile()
    _CACHE["nc"] = nc
    return nc


def _host_inputs(x, freqs_cos, freqs_sin, wq, wk, wv, wo):
    x = np.asarray(x, np.float32)
    freqs_cos = np.asarray(freqs_cos, np.float32)
    freqs_sin = np.asarray(freqs_sin, np.float32)
    wq = np.asarray(wq, np.float32)
    wk = np.asarray(wk, np.float32)
    wv = np.asarray(wv, np.float32)
    wo = np.asarray(wo, np.float32)

    # x_pre[p, n, r, sq] = x[512n+sq, 128r+p]
    xv = x[0].reshape(NQ, SQ, NR, 128)
    x_pre = np.ascontiguousarray(xv.transpose(3, 0, 2, 1)).astype(ml_dtypes.bfloat16)

    cosT = freqs_cos.T                                              # [32, 4096]
    sinT = freqs_sin.T
    cos4 = np.tile(cosT, (4, 1)).reshape(128, NQ, SQ)
    sin4 = np.concatenate([sinT, -sinT, sinT, -sinT], axis=0).reshape(128, NQ, SQ)
    trig = np.ascontiguousarray(np.stack([cos4, sin4], axis=2)).astype(
        ml_dtypes.bfloat16)                                         # [128, 8, 2, 512]

    # diagonal-block causal masks for delta in {0,128,256,384}
    p = np.arange(SK)[:, None]
    f = np.arange(SQ)[None, :]
    mask = np.concatenate(
        [np.where(SK * d + p <= f, 0.0, MASKVAL) for d in range(4)],
        axis=1).astype(ml_dtypes.bfloat16)                          # [128, 2048]

    ident = np.eye(128, dtype=ml_dtypes.bfloat16)
    sel2 = np.zeros((2, 128), dtype=np.float32)
    sel2[0, 0:64] = 1.0
    sel2[1, 64:128] = 1.0

    perm = np.concatenate([np.arange(0, HD, 2), np.arange(1, HD, 2)])

    def fold(w):  # [128(m), 1024(d)] -> lhsT layout [128(p), 8r*128+m]
        return np.ascontiguousarray(
            w.reshape(128, NR, 128).transpose(2, 1, 0).reshape(128, DIM)
        ).astype(ml_dtypes.bfloat16)

    in_maps = []
    for c in range(NCORES):
        g = c // 2
        wq_c = wq[128 * c:128 * (c + 1)].reshape(2, HD, DIM)[:, perm, :].reshape(128, DIM)
        wk_g = wk[HD * g:HD * (g + 1)][perm]
        wv_g = wv[HD * g:HD * (g + 1)]
        wkv_c = np.concatenate([wv_g, wk_g], axis=0)        # v rows 0:64, k rows 64:128
        wo_c = np.ascontiguousarray(wo[:, 128 * c:128 * (c + 1)].T).astype(
            ml_dtypes.bfloat16)                              # [128(j), 1024(o)]
        in_maps.append({
            "xT": x_pre,
            "wq_l": fold(wq_c),
            "wkv_l": fold(wkv_c),
            "wo_l": wo_c,
            "trig": trig,
            "mask": mask,
            "ident": ident,
            "sel2": sel2,
        })
    return in_maps


def kernel(x, freqs_cos, freqs_sin, wq, wk, wv, wo, _trace=False, _trace_kwargs=None):
    nc = _build()
    in_maps = _host_inputs(x, freqs_cos, freqs_sin, wq, wk, wv, wo)
    kw = {}
    if _trace:
        kw.update(trace=True, **(_trace_kwargs or {}))
    res = run_bass_kernel_spmd(nc, in_maps, core_ids=list(range(NCORES)), **kw)
    acc = np.zeros((128, NQ, NR, SQ), np.float32)
    for c in range(NCORES):
        acc += np.asarray(res.results[c]["out"], np.float32)
    # out[p, n, m, sq] -> [512n+sq, 128m+p]
    out = np.ascontiguousarray(acc.transpose(1, 3, 2, 0)).reshape(1, SEQ, DIM)
    if _trace:
        kernel._last_results = res
    return out
